# revision 1
# baseline (speedup 1.0000x reference)
"""Causal self-attention (B=1, T=4096, D=1024, H=16, HD=64) on 8 trn2 NeuronCores.

Sharding: tensor-parallel over heads (2 heads per core) for QKV + attention;
on-chip AllToAll re-shards to sequence-parallel for the output projection
(each core computes a 512-row slice of the output).

Matmul layout notes (PE computes out = lhsT.T @ rhs, contraction on partitions):
 - host feeds x transposed (xT [D, T]) so QKV needs no on-chip transposes.
 - S^T tiles [tk, tq] are computed (not S) so the PV matmul can consume
   exp(S^T) directly as the moving operand with V in natural [tk, hd] layout.
   The two heads' QK matmuls run concurrently via PE row tiling (K=64 each).
 - a ones-column appended to V makes row 64 of the PV accumulator the
   softmax denominator (no extra reduction pass).
 - softmax max-subtraction is skipped: scores are ~N(0,1) (|s| < ~10), and
   a constant shift cancels exactly in softmax, so exp is safe in fp32.
"""

import math
import sys
from contextlib import ExitStack

sys.path.insert(0, "/opt/trn_rl_repo")

import ml_dtypes
import numpy as np

import concourse.bass as bass  # noqa: F401  (bass types used via tile/bacc)
import concourse.mybir as mybir
import concourse.tile as tile
from concourse import bacc
from concourse.bass_utils import run_bass_kernel_spmd

B, T, D, H, HD = 1, 4096, 1024, 16, 64
NCORES = 8
HPC = H // NCORES          # heads per core = 2
E = HPC * HD               # per-core head width = 128
TQ = 512                   # tq block width
NB = T // TQ               # 8 tq blocks
CK = 128                   # tk chunk (partition dim of S^T tiles)
KD = D // 128              # 8 contraction chunks over D
NV = T // CK               # 32 tk chunks total
VW = HD + 1                # V tile width incl. ones column = 65

BF16 = mybir.dt.bfloat16
F32 = mybir.dt.float32
NPBF16 = ml_dtypes.bfloat16

_CACHE = {}


def _build():
    nc = bacc.Bacc("TRN2", target_bir_lowering=False, debug=False, num_devices=NCORES)
    xT = nc.dram_tensor("xT", [D, T], BF16, kind="ExternalInput").ap()
    wqT = nc.dram_tensor("wqT", [D, E], BF16, kind="ExternalInput").ap()
    wkT = nc.dram_tensor("wkT", [D, E], BF16, kind="ExternalInput").ap()
    wvT = nc.dram_tensor("wvT", [D, E], BF16, kind="ExternalInput").ap()
    wpT = nc.dram_tensor("wpT", [D, D], BF16, kind="ExternalInput").ap()
    trim = nc.dram_tensor("trim", [128, 128], BF16, kind="ExternalInput").ap()
    madd = nc.dram_tensor("madd", [128, 1024], BF16, kind="ExternalInput").ap()
    out = nc.dram_tensor("out", [TQ, D], F32, kind="ExternalOutput").ap()

    with tile.TileContext(nc) as tc, ExitStack() as ctx:
        sing = ctx.enter_context(tc.tile_pool(name="sing", bufs=1))
        pwork = ctx.enter_context(tc.tile_pool(name="pwork", bufs=3))
        ynp = ctx.enter_context(tc.tile_pool(name="ynp", bufs=4))
        osb = ctx.enter_context(tc.tile_pool(name="osb", bufs=2))
        # PSUM (8 banks): psS 3 x [128,1024] (2 banks) = 6 for S tiles and
        # QKV/proj accumulators; psY 2 x [65,512] (1 bank) = 2 for y0/y1.
        psS = ctx.enter_context(tc.tile_pool(name="psS", bufs=3, space="PSUM"))
        psY = ctx.enter_context(tc.tile_pool(name="psY", bufs=2, space="PSUM"))
        dram = ctx.enter_context(tc.tile_pool(name="dram", bufs=1, space="DRAM"))

        # ---- resident SBUF tensors -------------------------------------
        xT_sb = sing.tile([128, KD * T], BF16)      # d-chunk kc at cols [kc*T, (kc+1)*T)
        wq_sb = sing.tile([128, KD * E], BF16)
        wk_sb = sing.tile([128, KD * E], BF16)
        wv_sb = sing.tile([128, KD * E], BF16)
        wp_sb = sing.tile([128, KD * D], BF16)
        trim_sb = sing.tile([128, 128], BF16)       # trim[k,p] = 1{p>=k}
        madd_sb = sing.tile([128, 1024], BF16)      # shifted -1e9 diag + left fill
        qT_sb = sing.tile([128, T], BF16)           # rows 0:64 head0, 64:128 head1
        kT_sb = sing.tile([128, T], BF16)
        v0_sb = sing.tile([128, NV * VW], BF16)     # V head0 + ones col per chunk
        v1_sb = sing.tile([128, NV * VW], BF16)
        ya_sb = sing.tile([128, KD * TQ], BF16)     # gathered y^T for my tq rows
        y_sb = sing.tile([128, T], F32)             # unnormalized y^T (both heads)
        dsp_sb = sing.tile([128, 4 * TQ], BF16)     # den rows at partitions {0,32,64,96}
        onesp_sb = sing.tile([128, 128], BF16)
        zb_sb = sing.tile([128, 1], F32)            # zero bias for activations

        nc.vector.memset(zb_sb[:], 0.0)
        nc.vector.memset(onesp_sb[:], 1.0)
        nc.vector.memset(
            v0_sb[:].rearrange("p (c w) -> p c w", w=VW)[:, :, HD : HD + 1], 1.0
        )
        nc.vector.memset(
            v1_sb[:].rearrange("p (c w) -> p c w", w=VW)[:, :, HD : HD + 1], 1.0
        )

        # ---- input DMAs (weights first; xT t-slice-major) ---------------
        def load_w(w_sb, w_dram):
            nc.sync.dma_start(
                out=w_sb[:].rearrange("p (c e) -> p c e", c=KD),
                in_=w_dram.rearrange("(c p) e -> p c e", p=128),
            )

        def load_xt(tb):
            nc.sync.dma_start(
                out=xT_sb[:].rearrange("p (c t) -> p c t", c=KD)[
                    :, :, tb * TQ : (tb + 1) * TQ
                ],
                in_=xT[:, tb * TQ : (tb + 1) * TQ].rearrange("(c p) t -> p c t", p=128),
            )

        load_xt(0)
        load_w(wq_sb, wqT)
        load_w(wk_sb, wkT)
        load_w(wv_sb, wvT)
        nc.scalar.dma_start(out=trim_sb[:], in_=trim)
        nc.scalar.dma_start(out=madd_sb[:], in_=madd)
        for tb in range(1, NB):
            load_xt(tb)
        nc.scalar.dma_start(
            out=wp_sb[:].rearrange("p (c e) -> p c e", c=KD),
            in_=wpT.rearrange("(c p) e -> p c e", p=128),
        )

        # ---- QKV helpers (emitted lazily as PE filler) -------------------
        def emit_q(tb):
            ts = tb * TQ
            psq = psS.tile([128, 1024], F32, tag="ps", name=f"psq{tb}")
            for kc in range(KD):
                nc.tensor.matmul(
                    out=psq[:, 0:TQ],
                    lhsT=wq_sb[:, kc * E : (kc + 1) * E],
                    rhs=xT_sb[:, kc * T + ts : kc * T + ts + TQ],
                    start=(kc == 0),
                    stop=(kc == KD - 1),
                )
            nc.scalar.copy(out=qT_sb[:, ts : ts + TQ], in_=psq[:, 0:TQ])

        def emit_k(tb):
            ts = tb * TQ
            psk = psS.tile([128, 1024], F32, tag="ps", name=f"psk{tb}")
            for kc in range(KD):
                nc.tensor.matmul(
                    out=psk[:, 0:TQ],
                    lhsT=wk_sb[:, kc * E : (kc + 1) * E],
                    rhs=xT_sb[:, kc * T + ts : kc * T + ts + TQ],
                    start=(kc == 0),
                    stop=(kc == KD - 1),
                )
            nc.vector.tensor_copy(out=kT_sb[:, ts : ts + TQ], in_=psk[:, 0:TQ])

        def emit_v(ci):
            psv = psS.tile([128, 1024], F32, tag="ps", name=f"psv{ci}")
            for kc in range(KD):
                nc.tensor.matmul(
                    out=psv[:, 0:E],
                    lhsT=xT_sb[:, kc * T + ci * CK : kc * T + (ci + 1) * CK],
                    rhs=wv_sb[:, kc * E : (kc + 1) * E],
                    start=(kc == 0),
                    stop=(kc == KD - 1),
                )
            nc.vector.tensor_copy(
                out=v0_sb[:, ci * VW : ci * VW + HD], in_=psv[:, 0:HD]
            )
            nc.scalar.copy(out=v1_sb[:, ci * VW : ci * VW + HD], in_=psv[:, HD:E])

        send_t = dram.tile([NCORES, 128, TQ], BF16)
        recv_t = dram.tile([NCORES, 128, TQ], BF16)

        def emit_norm_send(b):
            # K=1 bf16 matmuls broadcast the denominator rows across
            # partitions, then 1/x and one multiply; ship block to its core.
            rbb = psS.tile([128, TQ], F32, tag="ps", name=f"rbb{b}")
            for h in range(2):
                i = 2 * b + h
                pr = (i % 4) * 32
                cr = (i // 4) * TQ
                nc.tensor.matmul(
                    out=rbb[h * HD : (h + 1) * HD, :],
                    lhsT=onesp_sb[pr : pr + 1, 0:HD],
                    rhs=dsp_sb[pr : pr + 1, cr : cr + TQ],
                    start=True,
                    stop=True,
                    tile_position=(pr, h * HD),
                )
            rq = ynp.tile([128, TQ], F32, tag="rq", name=f"rq{b}")
            nc.vector.reciprocal_approx_fast(out=rq[:], in_=rbb[:])
            yn = ynp.tile([128, TQ], BF16, tag="yn", name=f"yn{b}")
            nc.vector.tensor_mul(yn[:], y_sb[:, b * TQ : (b + 1) * TQ], rq[:])
            nc.sync.dma_start(out=send_t[b, :, :], in_=yn[:])

        # ---- attention: per tq block, with next-block QKV matmul groups
        # interleaved as PE filler (keeps TensorE dense so HAM stays warm).
        emit_q(0)
        emit_k(0)
        for ci in range(4):
            emit_v(ci)

        for b in range(NB):
            ts = b * TQ
            nchunks = 4 * (b + 1)
            nsc = nchunks // 2
            # filler: block b+1's QKV groups, spread across this block
            filler = []
            if b + 1 < NB:
                filler = (
                    [lambda tb=b + 1: emit_q(tb), lambda tb=b + 1: emit_k(tb)]
                    + [lambda ci=c: emit_v(ci) for c in range(4 * b + 4, 4 * b + 8)]
                )
            fidx = 0
            fevery = max(1, nsc // max(len(filler), 1))
            y0 = psY.tile([VW, TQ], F32, tag="py", name=f"y0_{b}")
            y1 = psY.tile([VW, TQ], F32, tag="py", name=f"y1_{b}")
            for sc in range(nsc):
                if b > 0 and sc == min(2, nsc - 1):
                    emit_norm_send(b - 1)
                s0 = psS.tile([128, 1024], F32, tag="ps", name=f"s0_{b}_{sc}")
                s1 = psS.tile([128, 1024], F32, tag="ps", name=f"s1_{b}_{sc}")
                for j in range(2):
                    ci = 2 * sc + j
                    off = ci * CK - ts
                    diag = off >= 0
                    nc.tensor.matmul(
                        out=s0[:, j * TQ : (j + 1) * TQ],
                        lhsT=kT_sb[0:HD, ci * CK : (ci + 1) * CK],
                        rhs=qT_sb[0:HD, ts : ts + TQ],
                        start=True,
                        stop=not diag,
                    )
                    nc.tensor.matmul(
                        out=s1[:, j * TQ : (j + 1) * TQ],
                        lhsT=kT_sb[HD:128, ci * CK : (ci + 1) * CK],
                        rhs=qT_sb[HD:128, ts : ts + TQ],
                        start=True,
                        stop=not diag,
                    )
                    if diag:
                        # accumulate -1e9 onto causally-invalid entries:
                        # (trim.T @ madd_slice)[p, j] = -1e9 * 1{j < p + off}
                        nm = min(off + CK, TQ)
                        ms = madd_sb[:, 512 - off : 512 - off + nm]
                        nc.tensor.matmul(
                            out=s0[:, j * TQ : j * TQ + nm],
                            lhsT=trim_sb[:],
                            rhs=ms,
                            start=False,
                            stop=True,
                        )
                        nc.tensor.matmul(
                            out=s1[:, j * TQ : j * TQ + nm],
                            lhsT=trim_sb[:],
                            rhs=ms,
                            start=False,
                            stop=True,
                        )
                p0 = pwork.tile([128, 1024], BF16, tag="pt", name=f"p0_{b}_{sc}")
                p1 = pwork.tile([128, 1024], BF16, tag="pt", name=f"p1_{b}_{sc}")
                nc.scalar.activation(
                    out=p0[:], in_=s0[:], func=mybir.ActivationFunctionType.Exp,
                    bias=zb_sb[:],
                )
                nc.scalar.activation(
                    out=p1[:], in_=s1[:], func=mybir.ActivationFunctionType.Exp,
                    bias=zb_sb[:],
                )
                for j in range(2):
                    ci = 2 * sc + j
                    nc.tensor.matmul(
                        out=y0[:],
                        lhsT=v0_sb[:, ci * VW : (ci + 1) * VW],
                        rhs=p0[:, j * TQ : (j + 1) * TQ],
                        start=(ci == 0),
                        stop=(ci == nchunks - 1),
                    )
                    nc.tensor.matmul(
                        out=y1[:],
                        lhsT=v1_sb[:, ci * VW : (ci + 1) * VW],
                        rhs=p1[:, j * TQ : (j + 1) * TQ],
                        start=(ci == 0),
                        stop=(ci == nchunks - 1),
                    )
                if filler and (sc + 1) % fevery == 0 and fidx < len(filler):
                    filler[fidx]()
                    fidx += 1
            while fidx < len(filler):
                filler[fidx]()
                fidx += 1
            # stash unnormalized y + denominator rows (normalization deferred
            # into the next block so it never stalls this pipeline)
            for h, yy in ((0, y0), (1, y1)):
                i = 2 * b + h
                nc.vector.tensor_copy(
                    out=dsp_sb[
                        (i % 4) * 32 : (i % 4) * 32 + 1,
                        (i // 4) * TQ : (i // 4 + 1) * TQ,
                    ],
                    in_=yy[HD : HD + 1, :],
                )
            nc.vector.tensor_copy(out=y_sb[0:HD, ts : ts + TQ], in_=y0[0:HD, :])
            nc.scalar.copy(out=y_sb[HD:128, ts : ts + TQ], in_=y1[0:HD, :])
        emit_norm_send(NB - 1)

        # ---- all-to-all + output projection -----------------------------
        nc.gpsimd.collective_compute(
            "AllToAll",
            mybir.AluOpType.bypass,
            replica_groups=[list(range(NCORES))],
            ins=[send_t[:].opt()],
            outs=[recv_t[:].opt()],
        )
        for j in range(NCORES):
            nc.sync.dma_start(
                out=ya_sb[:, j * TQ : (j + 1) * TQ], in_=recv_t[j, :, :]
            )

        for mt in range(TQ // 128):
            out_sb = osb.tile([128, D], F32, tag="o", name=f"osb{mt}")
            for nh in range(2):
                po = psS.tile([128, 1024], F32, tag="ps", name=f"po{mt}_{nh}")
                for kc in range(KD):
                    nc.tensor.matmul(
                        out=po[:, 0:512],
                        lhsT=ya_sb[:, kc * TQ + mt * 128 : kc * TQ + (mt + 1) * 128],
                        rhs=wp_sb[:, kc * D + nh * 512 : kc * D + (nh + 1) * 512],
                        start=(kc == 0),
                        stop=(kc == KD - 1),
                    )
                if nh == 0:
                    nc.scalar.copy(out=out_sb[:, 0:512], in_=po[:, 0:512])
                else:
                    nc.vector.tensor_copy(out=out_sb[:, 512:1024], in_=po[:, 0:512])
            nc.sync.dma_start(out=out[mt * 128 : (mt + 1) * 128, :], in_=out_sb[:])

    nc.compile()
    return nc


def _inputs(x, w_attn, w_proj):
    x = np.asarray(x, dtype=np.float32).reshape(T, D)
    w_attn = np.asarray(w_attn, dtype=np.float32)
    w_proj = np.asarray(w_proj, dtype=np.float32)

    xT_np = np.ascontiguousarray(x.T).astype(NPBF16)
    wpT_np = np.ascontiguousarray(w_proj.T).astype(NPBF16)
    scale = 1.0 / math.sqrt(HD)
    trim_np = np.triu(np.ones((128, 128), dtype=np.float32)).astype(NPBF16)
    madd_np = np.zeros((128, 1024), dtype=np.float32)
    for k in range(128):
        madd_np[k, 511 + k] = -1e9
    madd_np[0, 0:511] = -1e9
    madd_np = madd_np.astype(NPBF16)

    in_maps = []
    for core in range(NCORES):
        r0 = core * E
        in_maps.append(
            {
                "xT": xT_np,
                "wqT": np.ascontiguousarray((w_attn[r0 : r0 + E, :] * scale).T).astype(
                    NPBF16
                ),
                "wkT": np.ascontiguousarray(w_attn[D + r0 : D + r0 + E, :].T).astype(
                    NPBF16
                ),
                "wvT": np.ascontiguousarray(
                    w_attn[2 * D + r0 : 2 * D + r0 + E, :].T
                ).astype(NPBF16),
                "wpT": wpT_np,
                "trim": trim_np,
                "madd": madd_np,
            }
        )
    return in_maps


def kernel(x, w_attn, w_proj, _trace=False):
    if "nc" not in _CACHE:
        _CACHE["nc"] = _build()
    nc = _CACHE["nc"]
    in_maps = _inputs(x, w_attn, w_proj)
    res = run_bass_kernel_spmd(
        nc, in_maps, core_ids=list(range(NCORES)), trace=_trace
    )
    _CACHE["last_result"] = res
    full = np.concatenate([res.results[c]["out"] for c in range(NCORES)], axis=0)
    return full.reshape(B, T, D).astype(np.float32)



# revision 6
# speedup vs baseline: 1.0142x; 1.0142x over previous
"""Causal self-attention (B=1, T=4096, D=1024, H=16, HD=64) on 8 trn2 NeuronCores.

Sharding: tensor-parallel over heads (2 heads per core) for QKV + attention.
Output ownership is row-interleaved: core c owns query rows {512*b + 64*c + s}
so the head->sequence re-shard is done with EIGHT small per-block AllToAlls
(128KB each) that overlap attention compute, instead of one 1MB AllToAll at
the end.  The output projection for row-groups 0-2 runs while the last
collective is still in flight; only the final 128-row group is serial.

Matmul layout notes (PE computes out = lhsT.T @ rhs, contraction on partitions):
 - host feeds x pre-shaped (block-major) so every input DMA is contiguous.
 - S^T tiles [tk, tq] are computed (not S) so the PV matmul can consume
   exp(S^T) directly as the moving operand with V in natural [tk, hd] layout.
 - a ones-column appended to V makes row 64 of the PV accumulator the
   softmax denominator (no extra reduction pass).
 - causal masking inside diagonal chunks is an elementwise multiply of
   exp(S^T) by a 0/1 mask on GpSimd/Vector (cheaper than PE mask matmuls).
 - softmax max-subtraction is skipped: scores are ~N(0,1) (|s| < ~10), and
   a constant shift cancels exactly in softmax, so exp is safe in fp32.
"""

import math
import sys
from contextlib import ExitStack

sys.path.insert(0, "/opt/trn_rl_repo")

import ml_dtypes
import numpy as np

import concourse.bass as bass  # noqa: F401  (bass types used via tile/bacc)
import concourse.mybir as mybir
import concourse.tile as tile
from concourse import bacc
from concourse.bass_utils import run_bass_kernel_spmd

B, T, D, H, HD = 1, 4096, 1024, 16, 64
NCORES = 8
HPC = H // NCORES          # heads per core = 2
E = HPC * HD               # per-core head width = 128
TQ = 512                   # tq block width
NB = T // TQ               # 8 tq blocks
CK = 128                   # tk chunk (partition dim of S^T tiles)
KD = D // 128              # 8 contraction chunks over D
NV = T // CK               # 32 tk chunks total
VW = HD + 1                # V tile width incl. ones column = 65
RS = TQ // NCORES          # per-core row strip within a block = 64

BF16 = mybir.dt.bfloat16
F32 = mybir.dt.float32
NPBF16 = ml_dtypes.bfloat16

_CACHE = {}


def _build():
    nc = bacc.Bacc("TRN2", target_bir_lowering=False, debug=False, num_devices=NCORES)
    xTb = nc.dram_tensor("xTb", [128, NB * KD * TQ], BF16, kind="ExternalInput").ap()
    wqT = nc.dram_tensor("wqT", [128, KD * E], BF16, kind="ExternalInput").ap()
    wkT = nc.dram_tensor("wkT", [128, KD * E], BF16, kind="ExternalInput").ap()
    wvT = nc.dram_tensor("wvT", [128, KD * E], BF16, kind="ExternalInput").ap()
    wpT = nc.dram_tensor("wpT", [128, KD * D], BF16, kind="ExternalInput").ap()
    pmask = nc.dram_tensor("pmask", [128, 2048], BF16, kind="ExternalInput").ap()
    out = nc.dram_tensor("out", [TQ, D], F32, kind="ExternalOutput").ap()

    with tile.TileContext(nc) as tc, ExitStack() as ctx:
        sing = ctx.enter_context(tc.tile_pool(name="sing", bufs=1))
        pwork = ctx.enter_context(tc.tile_pool(name="pwork", bufs=3))
        ynp = ctx.enter_context(tc.tile_pool(name="ynp", bufs=4))
        osb = ctx.enter_context(tc.tile_pool(name="osb", bufs=2))
        # PSUM (8 banks): psS 3 x [128,1024] (2 banks) = 6 for S tiles and
        # QKV/proj accumulators; psY 2 x [65,512] (1 bank) = 2 for y0/y1.
        psS = ctx.enter_context(tc.tile_pool(name="psS", bufs=3, space="PSUM"))
        psY = ctx.enter_context(tc.tile_pool(name="psY", bufs=2, space="PSUM"))
        dram = ctx.enter_context(tc.tile_pool(name="dram", bufs=1, space="DRAM"))

        # ---- resident SBUF tensors -------------------------------------
        xT_sb = sing.tile([128, KD * T], BF16)      # d-chunk kc at cols [kc*T, (kc+1)*T)
        wq_sb = sing.tile([128, KD * E], BF16)
        wk_sb = sing.tile([128, KD * E], BF16)
        wv_sb = sing.tile([128, KD * E], BF16)
        wp_sb = sing.tile([128, KD * D], BF16)
        mask_sb = sing.tile([128, 2048], BF16)      # diag 0/1 masks (2 sc steps)
        qT_sb = sing.tile([128, T], BF16)           # rows 0:64 head0, 64:128 head1
        kT_sb = sing.tile([128, T], BF16)
        v0_sb = sing.tile([128, NV * VW], BF16)     # V head0 + ones col per chunk
        v1_sb = sing.tile([128, NV * VW], BF16)
        ya_sb = sing.tile([128, KD * TQ], BF16)     # gathered y^T for my rows
        y_sb = sing.tile([128, T], F32)             # unnormalized y^T (both heads)
        dsp_sb = sing.tile([128, 4 * TQ], BF16)     # den rows at partitions {0,32,64,96}
        onesp_sb = sing.tile([128, 128], BF16)
        zb_sb = sing.tile([128, 1], F32)            # zero bias for activations

        nc.vector.memset(zb_sb[:], 0.0)
        nc.vector.memset(onesp_sb[:], 1.0)
        nc.vector.memset(
            v0_sb[:].rearrange("p (c w) -> p c w", w=VW)[:, :, HD : HD + 1], 1.0
        )
        nc.vector.memset(
            v1_sb[:].rearrange("p (c w) -> p c w", w=VW)[:, :, HD : HD + 1], 1.0
        )

        # ---- input DMAs: contiguous source layouts, spread over queues --
        def load_xt(tb):
            # xTb block tb is [128, KD*TQ] contiguous (kc-major within block)
            nc.sync.dma_start(
                out=xT_sb[:].rearrange("p (c t) -> p c t", c=KD)[
                    :, :, tb * TQ : (tb + 1) * TQ
                ],
                in_=xTb[:, tb * (KD * TQ) : (tb + 1) * (KD * TQ)].rearrange(
                    "p (c t) -> p c t", c=KD
                ),
            )

        load_xt(0)
        nc.scalar.dma_start(out=wq_sb[:], in_=wqT)
        nc.scalar.dma_start(out=wk_sb[:], in_=wkT)
        nc.gpsimd.dma_start(out=wv_sb[:], in_=wvT)
        nc.gpsimd.dma_start(out=mask_sb[:], in_=pmask)
        nc.scalar.dma_start(out=wp_sb[:], in_=wpT)
        for tb in range(1, NB):
            load_xt(tb)

        # ---- QKV helpers (emitted lazily as PE filler) -------------------
        def emit_q(tb):
            ts = tb * TQ
            psq = psS.tile([128, 1024], F32, tag="ps", name=f"psq{tb}")
            for kc in range(KD):
                nc.tensor.matmul(
                    out=psq[:, 0:TQ],
                    lhsT=wq_sb[:, kc * E : (kc + 1) * E],
                    rhs=xT_sb[:, kc * T + ts : kc * T + ts + TQ],
                    start=(kc == 0),
                    stop=(kc == KD - 1),
                )
            nc.vector.tensor_copy(out=qT_sb[:, ts : ts + TQ], in_=psq[:, 0:TQ])

        def emit_k(tb):
            ts = tb * TQ
            psk = psS.tile([128, 1024], F32, tag="ps", name=f"psk{tb}")
            for kc in range(KD):
                nc.tensor.matmul(
                    out=psk[:, 0:TQ],
                    lhsT=wk_sb[:, kc * E : (kc + 1) * E],
                    rhs=xT_sb[:, kc * T + ts : kc * T + ts + TQ],
                    start=(kc == 0),
                    stop=(kc == KD - 1),
                )
            nc.vector.tensor_copy(out=kT_sb[:, ts : ts + TQ], in_=psk[:, 0:TQ])

        def emit_v(ci):
            psv = psS.tile([128, 1024], F32, tag="ps", name=f"psv{ci}")
            for kc in range(KD):
                nc.tensor.matmul(
                    out=psv[:, 0:E],
                    lhsT=xT_sb[:, kc * T + ci * CK : kc * T + (ci + 1) * CK],
                    rhs=wv_sb[:, kc * E : (kc + 1) * E],
                    start=(kc == 0),
                    stop=(kc == KD - 1),
                )
            nc.vector.tensor_copy(
                out=v0_sb[:, ci * VW : ci * VW + HD], in_=psv[:, 0:HD]
            )
            nc.vector.tensor_copy(
                out=v1_sb[:, ci * VW : ci * VW + HD], in_=psv[:, HD:E]
            )

        # per-block exchange buffers (send local, recv shared for cc perf)
        send_t = [
            dram.tile([NCORES, 128, RS], BF16, name=f"send{b}", tag=f"send{b}")
            for b in range(NB)
        ]
        recv_t = [
            dram.tile([NCORES, 128, RS], BF16, name=f"recv{b}", tag=f"recv{b}")
            for b in range(NB)
        ]

        def emit_recv(b):
            # recv piece j = dims of core j's heads for my 64 rows of block b
            nc.sync.dma_start(
                out=ya_sb[:].rearrange("p (j q) -> p j q", j=NCORES)[
                    :, :, b * RS : (b + 1) * RS
                ],
                in_=recv_t[b][:].rearrange("j p r -> p j r"),
            )

        def emit_norm_send(b):
            # K=1 bf16 matmuls broadcast the denominator rows across
            # partitions, then 1/x and one multiply; ship block b to its
            # per-row-strip owners via a small AllToAll.
            rbb = psS.tile([128, TQ], F32, tag="ps", name=f"rbb{b}")
            for h in range(2):
                i = 2 * b + h
                pr = (i % 4) * 32
                cr = (i // 4) * TQ
                nc.tensor.matmul(
                    out=rbb[h * HD : (h + 1) * HD, :],
                    lhsT=onesp_sb[pr : pr + 1, 0:HD],
                    rhs=dsp_sb[pr : pr + 1, cr : cr + TQ],
                    start=True,
                    stop=True,
                    tile_position=(pr, h * HD),
                )
            rq = ynp.tile([128, TQ], F32, tag="rq", name=f"rq{b}")
            nc.vector.reciprocal_approx_fast(out=rq[:], in_=rbb[:])
            yn = ynp.tile([128, TQ], BF16, tag="yn", name=f"yn{b}")
            nc.vector.tensor_mul(yn[:], y_sb[:, b * TQ : (b + 1) * TQ], rq[:])
            nc.sync.dma_start(
                out=send_t[b][:].rearrange("j p r -> p j r"),
                in_=yn[:].rearrange("p (j r) -> p j r", j=NCORES),
            )
            nc.gpsimd.collective_compute(
                "AllToAll",
                mybir.AluOpType.bypass,
                replica_groups=[list(range(NCORES))],
                ins=[send_t[b][:].opt()],
                outs=[recv_t[b][:].opt()],
            )
            if b >= 1:
                emit_recv(b - 1)

        # ---- attention: per tq block, with next-block QKV matmul groups
        # interleaved as PE filler (keeps TensorE dense so HAM stays warm).
        emit_q(0)
        emit_k(0)
        for ci in range(4):
            emit_v(ci)

        for b in range(NB):
            ts = b * TQ
            nchunks = 4 * (b + 1)
            nsc = nchunks // 2
            # filler: block b+1's QKV groups, spread across this block
            filler = []
            if b + 1 < NB:
                filler = (
                    [lambda tb=b + 1: emit_q(tb), lambda tb=b + 1: emit_k(tb)]
                    + [lambda ci=c: emit_v(ci) for c in range(4 * b + 4, 4 * b + 8)]
                )
            fidx = 0
            fevery = max(1, nsc // max(len(filler), 1))
            y0 = psY.tile([VW, TQ], F32, tag="py", name=f"y0_{b}")
            y1 = psY.tile([VW, TQ], F32, tag="py", name=f"y1_{b}")
            for sc in range(nsc):
                if b > 0 and sc == min(2, nsc - 1):
                    emit_norm_send(b - 1)
                s0 = psS.tile([128, 1024], F32, tag="ps", name=f"s0_{b}_{sc}")
                s1 = psS.tile([128, 1024], F32, tag="ps", name=f"s1_{b}_{sc}")
                for j in range(2):
                    ci = 2 * sc + j
                    nc.tensor.matmul(
                        out=s0[:, j * TQ : (j + 1) * TQ],
                        lhsT=kT_sb[0:HD, ci * CK : (ci + 1) * CK],
                        rhs=qT_sb[0:HD, ts : ts + TQ],
                        start=True,
                        stop=True,
                    )
                    nc.tensor.matmul(
                        out=s1[:, j * TQ : (j + 1) * TQ],
                        lhsT=kT_sb[HD:128, ci * CK : (ci + 1) * CK],
                        rhs=qT_sb[HD:128, ts : ts + TQ],
                        start=True,
                        stop=True,
                    )
                p0 = pwork.tile([128, 1024], BF16, tag="pt", name=f"p0_{b}_{sc}")
                p1 = pwork.tile([128, 1024], BF16, tag="pt", name=f"p1_{b}_{sc}")
                nc.scalar.activation(
                    out=p0[:], in_=s0[:], func=mybir.ActivationFunctionType.Exp,
                    bias=zb_sb[:],
                )
                nc.scalar.activation(
                    out=p1[:], in_=s1[:], func=mybir.ActivationFunctionType.Exp,
                    bias=zb_sb[:],
                )
                if sc >= nsc - 2:
                    # diagonal chunks: zero the causally-invalid entries
                    dm = mask_sb[:, (sc - (nsc - 2)) * 1024 : (sc - (nsc - 2) + 1) * 1024]
                    nc.gpsimd.tensor_mul(p0[:], p0[:], dm)
                    nc.vector.tensor_mul(p1[:], p1[:], dm)
                for j in range(2):
                    ci = 2 * sc + j
                    nc.tensor.matmul(
                        out=y0[:],
                        lhsT=v0_sb[:, ci * VW : (ci + 1) * VW],
                        rhs=p0[:, j * TQ : (j + 1) * TQ],
                        start=(ci == 0),
                        stop=(ci == nchunks - 1),
                    )
                    nc.tensor.matmul(
                        out=y1[:],
                        lhsT=v1_sb[:, ci * VW : (ci + 1) * VW],
                        rhs=p1[:, j * TQ : (j + 1) * TQ],
                        start=(ci == 0),
                        stop=(ci == nchunks - 1),
                    )
                if filler and (sc + 1) % fevery == 0 and fidx < len(filler):
                    filler[fidx]()
                    fidx += 1
            while fidx < len(filler):
                filler[fidx]()
                fidx += 1
            # stash unnormalized y + denominator rows (normalization deferred
            # into the next block so it never stalls this pipeline)
            for h, yy in ((0, y0), (1, y1)):
                i = 2 * b + h
                nc.vector.tensor_copy(
                    out=dsp_sb[
                        (i % 4) * 32 : (i % 4) * 32 + 1,
                        (i // 4) * TQ : (i // 4 + 1) * TQ,
                    ],
                    in_=yy[HD : HD + 1, :],
                )
            nc.vector.tensor_copy(out=y_sb[0:HD, ts : ts + TQ], in_=y0[0:HD, :])
            nc.vector.tensor_copy(out=y_sb[HD:128, ts : ts + TQ], in_=y1[0:HD, :])
        emit_norm_send(NB - 1)
        emit_recv(NB - 2)

        # ---- output projection; groups 0-2 run while the last AllToAll is
        # still in flight, only group 3 (rows of blocks 6,7) is serial.
        def emit_proj(mt):
            out_sb = osb.tile([128, D], F32, tag="o", name=f"osb{mt}")
            for nh in range(2):
                po = psS.tile([128, 1024], F32, tag="ps", name=f"po{mt}_{nh}")
                for kc in range(KD):
                    nc.tensor.matmul(
                        out=po[:, 0:512],
                        lhsT=ya_sb[:, kc * TQ + mt * 128 : kc * TQ + (mt + 1) * 128],
                        rhs=wp_sb[:, kc * D + nh * 512 : kc * D + (nh + 1) * 512],
                        start=(kc == 0),
                        stop=(kc == KD - 1),
                    )
                if nh == 0:
                    nc.scalar.copy(out=out_sb[:, 0:512], in_=po[:, 0:512])
                else:
                    nc.vector.tensor_copy(out=out_sb[:, 512:1024], in_=po[:, 0:512])
            nc.sync.dma_start(out=out[mt * 128 : (mt + 1) * 128, :], in_=out_sb[:])

        for mt in range(3):
            emit_proj(mt)
        emit_recv(NB - 1)
        emit_proj(3)

    nc.compile()
    return nc


def _inputs(x, w_attn, w_proj):
    x = np.asarray(x, dtype=np.float32).reshape(T, D)
    w_attn = np.asarray(w_attn, dtype=np.float32)
    w_proj = np.asarray(w_proj, dtype=np.float32)

    # xTb[p, tb, kc, tq] = x[tq + tb*TQ, kc*128 + p]  (block-major, contiguous loads)
    xT = x.T.reshape(KD, 128, NB, TQ)              # [kc, p, tb, tq]
    xTb_np = np.ascontiguousarray(xT.transpose(1, 2, 0, 3)).reshape(128, -1)
    xTb_np = xTb_np.astype(NPBF16)

    def wshape(w):  # [E_out rows, D] -> [128, KD*E_out] in kc-major SBUF layout
        wt = w.T.reshape(KD, 128, w.shape[0])      # [kc, p, e]
        return np.ascontiguousarray(wt.transpose(1, 0, 2)).reshape(128, -1).astype(
            NPBF16
        )

    wpT_np = wshape(w_proj)
    scale = 1.0 / math.sqrt(HD)

    # diag 0/1 masks: step s (0/1), chunk j (0/1) -> offset (2s+j)*CK
    pmask_np = np.zeros((128, 2048), dtype=np.float32)
    for s in range(2):
        for j in range(2):
            off = (2 * s + j) * CK
            for p in range(128):
                q0 = p + off
                if q0 < TQ:
                    pmask_np[p, s * 1024 + j * TQ + q0 : s * 1024 + (j + 1) * TQ] = 1.0
    pmask_np = pmask_np.astype(NPBF16)

    in_maps = []
    for core in range(NCORES):
        r0 = core * E
        in_maps.append(
            {
                "xTb": xTb_np,
                "wqT": wshape(w_attn[r0 : r0 + E, :] * scale),
                "wkT": wshape(w_attn[D + r0 : D + r0 + E, :]),
                "wvT": wshape(w_attn[2 * D + r0 : 2 * D + r0 + E, :]),
                "wpT": wpT_np,
                "pmask": pmask_np,
            }
        )
    return in_maps


def kernel(x, w_attn, w_proj, _trace=False):
    if "nc" not in _CACHE:
        _CACHE["nc"] = _build()
    nc = _CACHE["nc"]
    in_maps = _inputs(x, w_attn, w_proj)
    res = run_bass_kernel_spmd(
        nc, in_maps, core_ids=list(range(NCORES)), trace=_trace
    )
    _CACHE["last_result"] = res
    # core c's out rows are (block b, strip s) pairs: full row = 512b + 64c + s
    arr = np.stack([res.results[c]["out"] for c in range(NCORES)])  # [c, b*64+s, D]
    arr = arr.reshape(NCORES, NB, RS, D).transpose(1, 0, 2, 3)      # [b, c, s, D]
    return arr.reshape(B, T, D).astype(np.float32)


# revision 10
# speedup vs baseline: 1.2169x; 1.1998x over previous
"""Causal self-attention (B=1, T=4096, D=1024, H=16, HD=64) on 8 trn2 NeuronCores.

Sharding: tensor-parallel over heads (2 heads per core) for QKV + attention.
Output ownership is row-interleaved: core c owns query rows {512*b + 64*c + s}
so the head->sequence re-shard is done with EIGHT small per-block AllToAlls
(128KB each) that overlap attention compute, instead of one 1MB AllToAll at
the end.  The output projection for row-groups 0-2 runs while the last
collective is still in flight; only the final 128-row group is serial.

Matmul layout notes (PE computes out = lhsT.T @ rhs, contraction on partitions):
 - host feeds x pre-shaped (block-major) so every input DMA is contiguous.
 - S^T tiles [tk, tq] are computed (not S) so the PV matmul can consume
   exp(S^T) directly as the moving operand with V in natural [tk, hd] layout.
 - a ones-column appended to V makes row 64 of the PV accumulator the
   softmax denominator (no extra reduction pass).
 - causal masking inside diagonal chunks is an elementwise multiply of
   exp(S^T) by a 0/1 mask on GpSimd/Vector (cheaper than PE mask matmuls).
 - softmax max-subtraction is skipped: scores are ~N(0,1) (|s| < ~10), and
   a constant shift cancels exactly in softmax, so exp is safe in fp32.
"""

import math
import sys
from contextlib import ExitStack

sys.path.insert(0, "/opt/trn_rl_repo")

import ml_dtypes
import numpy as np

import concourse.bass as bass  # noqa: F401  (bass types used via tile/bacc)
import concourse.mybir as mybir
import concourse.tile as tile
from concourse import bacc
from concourse.bass_utils import run_bass_kernel_spmd

B, T, D, H, HD = 1, 4096, 1024, 16, 64
NCORES = 8
HPC = H // NCORES          # heads per core = 2
E = HPC * HD               # per-core head width = 128
TQ = 512                   # tq block width
NB = T // TQ               # 8 tq blocks
CK = 128                   # tk chunk (partition dim of S^T tiles)
KD = D // 128              # 8 contraction chunks over D
NV = T // CK               # 32 tk chunks total
VW = HD + 1                # V tile width incl. ones column = 65
RS = TQ // NCORES          # per-core row strip within a block = 64

BF16 = mybir.dt.bfloat16
F32 = mybir.dt.float32
NPBF16 = ml_dtypes.bfloat16

_CACHE = {}


def _build():
    nc = bacc.Bacc("TRN2", target_bir_lowering=False, debug=False, num_devices=NCORES)
    xTb = nc.dram_tensor("xTb", [128, NB * KD * TQ], BF16, kind="ExternalInput").ap()
    wqT = nc.dram_tensor("wqT", [128, KD * E], BF16, kind="ExternalInput").ap()
    wkT = nc.dram_tensor("wkT", [128, KD * E], BF16, kind="ExternalInput").ap()
    wvT = nc.dram_tensor("wvT", [128, KD * E], BF16, kind="ExternalInput").ap()
    wpT = nc.dram_tensor("wpT", [128, KD * D], BF16, kind="ExternalInput").ap()
    pmask = nc.dram_tensor("pmask", [128, 2048], BF16, kind="ExternalInput").ap()
    out = nc.dram_tensor("out", [TQ, D], F32, kind="ExternalOutput").ap()

    with tile.TileContext(nc) as tc, ExitStack() as ctx:
        sing = ctx.enter_context(tc.tile_pool(name="sing", bufs=1))
        pwork = ctx.enter_context(tc.tile_pool(name="pwork", bufs=3))
        ynp = ctx.enter_context(tc.tile_pool(name="ynp", bufs=4))
        osb = ctx.enter_context(tc.tile_pool(name="osb", bufs=2))
        # PSUM (8 banks): psS 3 x [128,1024] (2 banks) = 6 for S tiles and
        # QKV/proj accumulators; psY 2 x [65,512] (1 bank) = 2 for y0/y1.
        psS = ctx.enter_context(tc.tile_pool(name="psS", bufs=3, space="PSUM"))
        psY = ctx.enter_context(tc.tile_pool(name="psY", bufs=2, space="PSUM"))
        dram = ctx.enter_context(tc.tile_pool(name="dram", bufs=1, space="DRAM"))

        # ---- resident SBUF tensors -------------------------------------
        xT_sb = sing.tile([128, KD * T], BF16)      # d-chunk kc at cols [kc*T, (kc+1)*T)
        wq_sb = sing.tile([128, KD * E], BF16)
        wk_sb = sing.tile([128, KD * E], BF16)
        wv_sb = sing.tile([128, KD * E], BF16)
        wp_sb = sing.tile([128, KD * D], BF16)
        mask_sb = sing.tile([128, 2048], BF16)      # diag 0/1 masks (2 sc steps)
        qT_sb = sing.tile([128, T], BF16)           # rows 0:64 head0, 64:128 head1
        kT_sb = sing.tile([128, T], BF16)
        # V both heads: chunk ci at cols [ci*2*VW, (ci+1)*2*VW): [v0|1][v1|1]
        v01_sb = sing.tile([128, NV * 2 * VW], BF16)
        ya_sb = sing.tile([128, KD * TQ], BF16)     # gathered y^T for my rows
        y_sb = sing.tile([128, T], F32)             # unnormalized y^T (both heads)
        dsp_sb = sing.tile([128, 4 * TQ], BF16)     # den rows at partitions {0,32,64,96}
        onesp_sb = sing.tile([128, 128], BF16)
        zb_sb = sing.tile([128, 1], F32)            # zero bias for activations

        nc.vector.memset(zb_sb[:], 0.0)
        nc.vector.memset(onesp_sb[:], 1.0)
        nc.vector.memset(
            v01_sb[:].rearrange("p (c w) -> p c w", w=VW)[:, :, HD : HD + 1], 1.0
        )

        # ---- input DMAs: contiguous source layouts, spread over queues --
        def load_xt(tb):
            # xTb block tb is [128, KD*TQ] contiguous (kc-major within block)
            nc.sync.dma_start(
                out=xT_sb[:].rearrange("p (c t) -> p c t", c=KD)[
                    :, :, tb * TQ : (tb + 1) * TQ
                ],
                in_=xTb[:, tb * (KD * TQ) : (tb + 1) * (KD * TQ)].rearrange(
                    "p (c t) -> p c t", c=KD
                ),
            )

        load_xt(0)
        nc.scalar.dma_start(out=wq_sb[:], in_=wqT)
        nc.scalar.dma_start(out=wk_sb[:], in_=wkT)
        nc.gpsimd.dma_start(out=wv_sb[:], in_=wvT)
        nc.gpsimd.dma_start(out=mask_sb[:], in_=pmask)
        nc.scalar.dma_start(out=wp_sb[:], in_=wpT)
        for tb in range(1, NB):
            load_xt(tb)

        # ---- QKV helpers (emitted lazily as PE filler) -------------------
        def emit_q(tb):
            ts = tb * TQ
            psq = psS.tile([128, 1024], F32, tag="ps", name=f"psq{tb}")
            for kc in range(KD):
                nc.tensor.matmul(
                    out=psq[:, 0:TQ],
                    lhsT=wq_sb[:, kc * E : (kc + 1) * E],
                    rhs=xT_sb[:, kc * T + ts : kc * T + ts + TQ],
                    start=(kc == 0),
                    stop=(kc == KD - 1),
                )
            nc.vector.tensor_copy(out=qT_sb[:, ts : ts + TQ], in_=psq[:, 0:TQ])

        def emit_k(tb):
            ts = tb * TQ
            psk = psS.tile([128, 1024], F32, tag="ps", name=f"psk{tb}")
            for kc in range(KD):
                nc.tensor.matmul(
                    out=psk[:, 0:TQ],
                    lhsT=wk_sb[:, kc * E : (kc + 1) * E],
                    rhs=xT_sb[:, kc * T + ts : kc * T + ts + TQ],
                    start=(kc == 0),
                    stop=(kc == KD - 1),
                )
            nc.vector.tensor_copy(out=kT_sb[:, ts : ts + TQ], in_=psk[:, 0:TQ])

        def emit_v(ci):
            psv = psS.tile([128, 1024], F32, tag="ps", name=f"psv{ci}")
            for kc in range(KD):
                nc.tensor.matmul(
                    out=psv[:, 0:E],
                    lhsT=xT_sb[:, kc * T + ci * CK : kc * T + (ci + 1) * CK],
                    rhs=wv_sb[:, kc * E : (kc + 1) * E],
                    start=(kc == 0),
                    stop=(kc == KD - 1),
                )
            nc.vector.tensor_copy(
                out=v01_sb[:].rearrange("p (c h w) -> p c h w", h=2, w=VW)[
                    :, ci, :, 0:HD
                ],
                in_=psv[:, 0:E].rearrange("p (h w) -> p h w", h=2),
            )

        # per-block exchange buffers (send local, recv shared for cc perf)
        send_t = [
            dram.tile([NCORES, 128, RS], BF16, name=f"send{b}", tag=f"send{b}")
            for b in range(NB)
        ]
        recv_t = [
            dram.tile([NCORES, 128, RS], BF16, name=f"recv{b}", tag=f"recv{b}")
            for b in range(NB)
        ]

        def emit_recv(b):
            # recv piece j = dims of core j's heads for my 64 rows of block b
            nc.sync.dma_start(
                out=ya_sb[:].rearrange("p (j q) -> p j q", j=NCORES)[
                    :, :, b * RS : (b + 1) * RS
                ],
                in_=recv_t[b][:].rearrange("j p r -> p j r"),
            )

        def emit_norm_send(b):
            # K=1 bf16 matmuls broadcast the denominator rows across
            # partitions, then 1/x and one multiply; ship block b to its
            # per-row-strip owners via a small AllToAll.
            rbb = psS.tile([128, TQ], F32, tag="ps", name=f"rbb{b}")
            for h in range(2):
                i = 2 * b + h
                pr = (i % 4) * 32
                cr = (i // 4) * TQ
                nc.tensor.matmul(
                    out=rbb[h * HD : (h + 1) * HD, :],
                    lhsT=onesp_sb[pr : pr + 1, 0:HD],
                    rhs=dsp_sb[pr : pr + 1, cr : cr + TQ],
                    start=True,
                    stop=True,
                    tile_position=(pr, h * HD),
                )
            rq = ynp.tile([128, TQ], F32, tag="rq", name=f"rq{b}")
            nc.vector.reciprocal_approx_fast(out=rq[:], in_=rbb[:])
            yn = ynp.tile([128, TQ], BF16, tag="yn", name=f"yn{b}")
            nc.vector.tensor_mul(yn[:], y_sb[:, b * TQ : (b + 1) * TQ], rq[:])
            nc.sync.dma_start(
                out=send_t[b][:].rearrange("j p r -> p j r"),
                in_=yn[:].rearrange("p (j r) -> p j r", j=NCORES),
            )
            nc.gpsimd.collective_compute(
                "AllToAll",
                mybir.AluOpType.bypass,
                replica_groups=[list(range(NCORES))],
                ins=[send_t[b][:].opt()],
                outs=[recv_t[b][:].opt()],
            )
            if b >= 1:
                emit_recv(b - 1)

        # ---- attention: per tq block, with next-block QKV matmul groups
        # interleaved as PE filler (keeps TensorE dense so HAM stays warm).
        emit_q(0)
        emit_k(0)
        for ci in range(4):
            emit_v(ci)

        for b in range(NB):
            ts = b * TQ
            nchunks = 4 * (b + 1)
            nsc = nchunks // 2
            # filler: block b+1's QKV groups, spread across this block
            filler = []
            if b + 1 < NB:
                filler = (
                    [lambda tb=b + 1: emit_q(tb), lambda tb=b + 1: emit_k(tb)]
                    + [lambda ci=c: emit_v(ci) for c in range(4 * b + 4, 4 * b + 8)]
                )
            fidx = 0
            fevery = max(1, nsc // max(len(filler), 1))
            y0 = psY.tile([VW, TQ], F32, tag="py", name=f"y0_{b}")
            y1 = psY.tile([VW, TQ], F32, tag="py", name=f"y1_{b}")
            # diagonal chunk-pairs first so the DVE mask-mul latency hides
            # under the history chunks instead of stalling the block boundary
            sc_order = [nsc - 2, nsc - 1] + list(range(nsc - 2)) if nsc >= 2 else [0]
            for si, sc in enumerate(sc_order):
                if b > 0 and si == min(2, nsc - 1):
                    emit_norm_send(b - 1)
                s0 = psS.tile([128, 1024], F32, tag="ps", name=f"s0_{b}_{sc}")
                s1 = psS.tile([128, 1024], F32, tag="ps", name=f"s1_{b}_{sc}")
                for j in range(2):
                    ci = 2 * sc + j
                    nc.tensor.matmul(
                        out=s0[:, j * TQ : (j + 1) * TQ],
                        lhsT=kT_sb[0:HD, ci * CK : (ci + 1) * CK],
                        rhs=qT_sb[0:HD, ts : ts + TQ],
                        start=True,
                        stop=True,
                    )
                    nc.tensor.matmul(
                        out=s1[:, j * TQ : (j + 1) * TQ],
                        lhsT=kT_sb[HD:128, ci * CK : (ci + 1) * CK],
                        rhs=qT_sb[HD:128, ts : ts + TQ],
                        start=True,
                        stop=True,
                    )
                p0 = pwork.tile([128, 1024], BF16, tag="pt", name=f"p0_{b}_{sc}")
                p1 = pwork.tile([128, 1024], BF16, tag="pt", name=f"p1_{b}_{sc}")
                nc.scalar.activation(
                    out=p0[:], in_=s0[:], func=mybir.ActivationFunctionType.Exp,
                    bias=zb_sb[:],
                )
                nc.scalar.activation(
                    out=p1[:], in_=s1[:], func=mybir.ActivationFunctionType.Exp,
                    bias=zb_sb[:],
                )
                if sc >= nsc - 2:
                    # diagonal chunks: zero the causally-invalid entries
                    dm = mask_sb[:, (sc - (nsc - 2)) * 1024 : (sc - (nsc - 2) + 1) * 1024]
                    nc.vector.tensor_mul(p0[:], p0[:], dm)
                    nc.vector.tensor_mul(p1[:], p1[:], dm)
                for j in range(2):
                    ci = 2 * sc + j
                    nc.tensor.matmul(
                        out=y0[:],
                        lhsT=v01_sb[:, ci * 2 * VW : ci * 2 * VW + VW],
                        rhs=p0[:, j * TQ : (j + 1) * TQ],
                        start=(si == 0 and j == 0),
                        stop=(si == nsc - 1 and j == 1),
                    )
                    nc.tensor.matmul(
                        out=y1[:],
                        lhsT=v01_sb[:, ci * 2 * VW + VW : (ci + 1) * 2 * VW],
                        rhs=p1[:, j * TQ : (j + 1) * TQ],
                        start=(si == 0 and j == 0),
                        stop=(si == nsc - 1 and j == 1),
                    )
                if filler and (si + 1) % fevery == 0 and fidx < len(filler):
                    filler[fidx]()
                    fidx += 1
            while fidx < len(filler):
                filler[fidx]()
                fidx += 1
            # stash unnormalized y + denominator rows (normalization deferred
            # into the next block so it never stalls this pipeline)
            for h, yy in ((0, y0), (1, y1)):
                i = 2 * b + h
                nc.vector.tensor_copy(
                    out=dsp_sb[
                        (i % 4) * 32 : (i % 4) * 32 + 1,
                        (i // 4) * TQ : (i // 4 + 1) * TQ,
                    ],
                    in_=yy[HD : HD + 1, :],
                )
            nc.vector.tensor_copy(out=y_sb[0:HD, ts : ts + TQ], in_=y0[0:HD, :])
            nc.vector.tensor_copy(out=y_sb[HD:128, ts : ts + TQ], in_=y1[0:HD, :])
        emit_norm_send(NB - 1)
        emit_recv(NB - 2)

        # ---- output projection; groups 0-2 run while the last AllToAll is
        # still in flight, only group 3 (rows of blocks 6,7) is serial.
        def emit_proj(mt):
            out_sb = osb.tile([128, D], F32, tag="o", name=f"osb{mt}")
            for nh in range(2):
                po = psS.tile([128, 1024], F32, tag="ps", name=f"po{mt}_{nh}")
                for kc in range(KD):
                    nc.tensor.matmul(
                        out=po[:, 0:512],
                        lhsT=ya_sb[:, kc * TQ + mt * 128 : kc * TQ + (mt + 1) * 128],
                        rhs=wp_sb[:, kc * D + nh * 512 : kc * D + (nh + 1) * 512],
                        start=(kc == 0),
                        stop=(kc == KD - 1),
                    )
                if nh == 0:
                    nc.scalar.copy(out=out_sb[:, 0:512], in_=po[:, 0:512])
                else:
                    nc.vector.tensor_copy(out=out_sb[:, 512:1024], in_=po[:, 0:512])
            nc.sync.dma_start(out=out[mt * 128 : (mt + 1) * 128, :], in_=out_sb[:])

        for mt in range(3):
            emit_proj(mt)
        emit_recv(NB - 1)
        emit_proj(3)

    nc.compile()
    return nc


def _inputs(x, w_attn, w_proj):
    x = np.asarray(x, dtype=np.float32).reshape(T, D)
    w_attn = np.asarray(w_attn, dtype=np.float32)
    w_proj = np.asarray(w_proj, dtype=np.float32)

    # xTb[p, tb, kc, tq] = x[tq + tb*TQ, kc*128 + p]  (block-major, contiguous loads)
    xT = x.T.reshape(KD, 128, NB, TQ)              # [kc, p, tb, tq]
    xTb_np = np.ascontiguousarray(xT.transpose(1, 2, 0, 3)).reshape(128, -1)
    xTb_np = xTb_np.astype(NPBF16)

    def wshape(w):  # [E_out rows, D] -> [128, KD*E_out] in kc-major SBUF layout
        wt = w.T.reshape(KD, 128, w.shape[0])      # [kc, p, e]
        return np.ascontiguousarray(wt.transpose(1, 0, 2)).reshape(128, -1).astype(
            NPBF16
        )

    wpT_np = wshape(w_proj)
    scale = 1.0 / math.sqrt(HD)

    # diag 0/1 masks: step s (0/1), chunk j (0/1) -> offset (2s+j)*CK
    pmask_np = np.zeros((128, 2048), dtype=np.float32)
    for s in range(2):
        for j in range(2):
            off = (2 * s + j) * CK
            for p in range(128):
                q0 = p + off
                if q0 < TQ:
                    pmask_np[p, s * 1024 + j * TQ + q0 : s * 1024 + (j + 1) * TQ] = 1.0
    pmask_np = pmask_np.astype(NPBF16)

    in_maps = []
    for core in range(NCORES):
        r0 = core * E
        in_maps.append(
            {
                "xTb": xTb_np,
                "wqT": wshape(w_attn[r0 : r0 + E, :] * scale),
                "wkT": wshape(w_attn[D + r0 : D + r0 + E, :]),
                "wvT": wshape(w_attn[2 * D + r0 : 2 * D + r0 + E, :]),
                "wpT": wpT_np,
                "pmask": pmask_np,
            }
        )
    return in_maps


def kernel(x, w_attn, w_proj, _trace=False):
    if "nc" not in _CACHE:
        _CACHE["nc"] = _build()
    nc = _CACHE["nc"]
    in_maps = _inputs(x, w_attn, w_proj)
    res = run_bass_kernel_spmd(
        nc, in_maps, core_ids=list(range(NCORES)), trace=_trace
    )
    _CACHE["last_result"] = res
    # core c's out rows are (block b, strip s) pairs: full row = 512b + 64c + s
    arr = np.stack([res.results[c]["out"] for c in range(NCORES)])  # [c, b*64+s, D]
    arr = arr.reshape(NCORES, NB, RS, D).transpose(1, 0, 2, 3)      # [b, c, s, D]
    return arr.reshape(B, T, D).astype(np.float32)


# revision 20
# speedup vs baseline: 1.2309x; 1.0115x over previous
"""Causal self-attention (B=1, T=4096, D=1024, H=16, HD=64) on 8 trn2 NeuronCores.

Sharding: tensor-parallel over heads (2 heads per core) for QKV + attention.
Output ownership is row-interleaved: core c owns query rows {512*b + 64*c + s}
so the head->sequence re-shard is done with EIGHT small per-block AllToAlls
(128KB each) that overlap attention compute, instead of one 1MB AllToAll at
the end.  The output projection for row-groups 0-2 runs while the last
collective is still in flight; only the final 128-row group is serial.

Matmul layout notes (PE computes out = lhsT.T @ rhs, contraction on partitions):
 - host feeds x pre-shaped (block-major) so every input DMA is contiguous.
 - S^T tiles [tk, tq] are computed (not S) so the PV matmul can consume
   exp(S^T) directly as the moving operand with V in natural [tk, hd] layout.
 - a ones-column appended to V makes row 64 of the PV accumulator the
   softmax denominator (no extra reduction pass).
 - causal masking inside diagonal chunks is an elementwise multiply of
   exp(S^T) by a 0/1 mask on GpSimd/Vector (cheaper than PE mask matmuls).
 - softmax max-subtraction is skipped: scores are ~N(0,1) (|s| < ~10), and
   a constant shift cancels exactly in softmax, so exp is safe in fp32.
"""

import math
import sys
from contextlib import ExitStack

sys.path.insert(0, "/opt/trn_rl_repo")

import ml_dtypes
import numpy as np

import concourse.bass as bass  # noqa: F401  (bass types used via tile/bacc)
import concourse.mybir as mybir
import concourse.tile as tile
from concourse import bacc
from concourse.bass_utils import run_bass_kernel_spmd

B, T, D, H, HD = 1, 4096, 1024, 16, 64
NCORES = 8
HPC = H // NCORES          # heads per core = 2
E = HPC * HD               # per-core head width = 128
TQ = 512                   # tq block width
NB = T // TQ               # 8 tq blocks
CK = 128                   # tk chunk (partition dim of S^T tiles)
KD = D // 128              # 8 contraction chunks over D
NV = T // CK               # 32 tk chunks total
VW = HD + 1                # V tile width incl. ones column = 65
RS = TQ // NCORES          # per-core row strip within a block = 64

BF16 = mybir.dt.bfloat16
F32 = mybir.dt.float32
NPBF16 = ml_dtypes.bfloat16

_CACHE = {}


def _build():
    nc = bacc.Bacc("TRN2", target_bir_lowering=False, debug=False, num_devices=NCORES)
    xTb = nc.dram_tensor("xTb", [128, NB * KD * TQ], BF16, kind="ExternalInput").ap()
    wqT = nc.dram_tensor("wqT", [128, KD * E], BF16, kind="ExternalInput").ap()
    wkT = nc.dram_tensor("wkT", [128, KD * E], BF16, kind="ExternalInput").ap()
    wvT = nc.dram_tensor("wvT", [128, KD * E], BF16, kind="ExternalInput").ap()
    wpT = nc.dram_tensor("wpT", [128, KD * D], BF16, kind="ExternalInput").ap()
    pmask = nc.dram_tensor("pmask", [128, 128], BF16, kind="ExternalInput").ap()
    out = nc.dram_tensor("out", [TQ, D], F32, kind="ExternalOutput").ap()

    with tile.TileContext(nc) as tc, ExitStack() as ctx:
        sing = ctx.enter_context(tc.tile_pool(name="sing", bufs=1))
        pwork = ctx.enter_context(tc.tile_pool(name="pwork", bufs=3))
        ynp = ctx.enter_context(tc.tile_pool(name="ynp", bufs=4))
        osb = ctx.enter_context(tc.tile_pool(name="osb", bufs=2))
        # PSUM (8 banks): psS 3 x [128,1024] (2 banks) = 6 for S tiles and
        # QKV/proj accumulators; psY 2 x [65,512] (1 bank) = 2 for y0/y1.
        psS = ctx.enter_context(tc.tile_pool(name="psS", bufs=3, space="PSUM"))
        psY = ctx.enter_context(tc.tile_pool(name="psY", bufs=2, space="PSUM"))
        dram = ctx.enter_context(tc.tile_pool(name="dram", bufs=1, space="DRAM"))

        # ---- resident SBUF tensors -------------------------------------
        xT_sb = sing.tile([128, KD * T], BF16)      # d-chunk kc at cols [kc*T, (kc+1)*T)
        wq_sb = sing.tile([128, KD * E], BF16)
        wk_sb = sing.tile([128, KD * E], BF16)
        wv_sb = sing.tile([128, KD * E], BF16)
        wp_sb = sing.tile([128, KD * D], BF16)
        tri_sb = sing.tile([128, 128], BF16)        # tri[p,q] = 1{q >= p}
        qT_sb = sing.tile([128, T], BF16)           # rows 0:64 head0, 64:128 head1
        kT_sb = sing.tile([128, T], BF16)
        # V both heads: chunk ci at cols [ci*2*VW, (ci+1)*2*VW): [v0|1][v1|1]
        v01_sb = sing.tile([128, NV * 2 * VW], BF16)
        # gathered y^T for my rows, one tile per 128-row proj group so the
        # projection's dependencies are exact (group mt <- recvs 2mt, 2mt+1)
        ya_mt = [
            sing.tile([128, KD * 128], BF16, name=f"ya{m}") for m in range(4)
        ]
        y_sb = sing.tile([128, T], F32)             # unnormalized y^T (both heads)
        dsp_sb = sing.tile([128, 4 * TQ], BF16)     # den rows at partitions {0,32,64,96}
        onesp_sb = sing.tile([128, 128], BF16)
        zb_sb = sing.tile([128, 1], F32)            # zero bias for activations

        nc.vector.memset(zb_sb[:], 0.0)
        nc.vector.memset(onesp_sb[:], 1.0)
        nc.vector.memset(
            v01_sb[:].rearrange("p (c w) -> p c w", w=VW)[:, :, HD : HD + 1], 1.0
        )

        # ---- input DMAs: contiguous source layouts, spread over queues --
        def load_xt(tb, kc=None, eng=None):
            # xTb block tb is [128, KD*TQ] contiguous (kc-major within block)
            cs = slice(0, KD) if kc is None else slice(kc, kc + 1)
            (eng or nc.sync).dma_start(
                out=xT_sb[:].rearrange("p (c t) -> p c t", c=KD)[
                    :, cs, tb * TQ : (tb + 1) * TQ
                ],
                in_=xTb[:, tb * (KD * TQ) : (tb + 1) * (KD * TQ)].rearrange(
                    "p (c t) -> p c t", c=KD
                )[:, cs, :],
            )

        # block 0 chunk-by-chunk across two queues so the first QKV matmuls
        # can start as soon as chunk 0 + wq land
        nc.scalar.dma_start(out=wq_sb[:], in_=wqT)
        for kc in range(KD):
            load_xt(0, kc=kc, eng=(nc.sync if kc % 2 == 0 else nc.scalar))
        nc.scalar.dma_start(out=wk_sb[:], in_=wkT)
        nc.gpsimd.dma_start(out=wv_sb[:], in_=wvT)
        nc.gpsimd.dma_start(out=tri_sb[:], in_=pmask)
        nc.scalar.dma_start(out=wp_sb[:], in_=wpT)
        for tb in range(1, NB):
            load_xt(tb)

        # ---- QKV helpers (emitted lazily as PE filler) -------------------
        def emit_q(tb):
            ts = tb * TQ
            psq = psS.tile([128, 1024], F32, tag="ps", name=f"psq{tb}")
            for kc in range(KD):
                nc.tensor.matmul(
                    out=psq[:, 0:TQ],
                    lhsT=wq_sb[:, kc * E : (kc + 1) * E],
                    rhs=xT_sb[:, kc * T + ts : kc * T + ts + TQ],
                    start=(kc == 0),
                    stop=(kc == KD - 1),
                )
            nc.vector.tensor_copy(out=qT_sb[:, ts : ts + TQ], in_=psq[:, 0:TQ])

        def emit_k(tb):
            ts = tb * TQ
            psk = psS.tile([128, 1024], F32, tag="ps", name=f"psk{tb}")
            for kc in range(KD):
                nc.tensor.matmul(
                    out=psk[:, 0:TQ],
                    lhsT=wk_sb[:, kc * E : (kc + 1) * E],
                    rhs=xT_sb[:, kc * T + ts : kc * T + ts + TQ],
                    start=(kc == 0),
                    stop=(kc == KD - 1),
                )
            nc.vector.tensor_copy(out=kT_sb[:, ts : ts + TQ], in_=psk[:, 0:TQ])

        def emit_v(ci):
            psv = psS.tile([128, 1024], F32, tag="ps", name=f"psv{ci}")
            for kc in range(KD):
                nc.tensor.matmul(
                    out=psv[:, 0:E],
                    lhsT=xT_sb[:, kc * T + ci * CK : kc * T + (ci + 1) * CK],
                    rhs=wv_sb[:, kc * E : (kc + 1) * E],
                    start=(kc == 0),
                    stop=(kc == KD - 1),
                )
            nc.vector.tensor_copy(
                out=v01_sb[:].rearrange("p (c h w) -> p c h w", h=2, w=VW)[
                    :, ci, :, 0:HD
                ],
                in_=psv[:, 0:E].rearrange("p (h w) -> p h w", h=2),
            )

        # per-block exchange buffers (send local, recv shared for cc perf)
        send_t = [
            dram.tile([NCORES, 128, RS], BF16, name=f"send{b}", tag=f"send{b}")
            for b in range(NB)
        ]
        recv_t = [
            dram.tile([NCORES, 128, RS], BF16, name=f"recv{b}", tag=f"recv{b}")
            for b in range(NB)
        ]

        def emit_recv(b):
            # recv piece j = dims of core j's heads for my 64 rows of block b
            nc.sync.dma_start(
                out=ya_mt[b // 2][:].rearrange("p (j g) -> p j g", j=NCORES)[
                    :, :, (b % 2) * RS : (b % 2 + 1) * RS
                ],
                in_=recv_t[b][:].rearrange("j p r -> p j r"),
            )

        def emit_norm_send(b):
            # K=1 bf16 matmuls broadcast the denominator rows across
            # partitions, then 1/x and one multiply; ship block b to its
            # per-row-strip owners via a small AllToAll.
            rbb = psS.tile([128, TQ], F32, tag="ps", name=f"rbb{b}")
            for h in range(2):
                i = 2 * b + h
                pr = (i % 4) * 32
                cr = (i // 4) * TQ
                nc.tensor.matmul(
                    out=rbb[h * HD : (h + 1) * HD, :],
                    lhsT=onesp_sb[pr : pr + 1, 0:HD],
                    rhs=dsp_sb[pr : pr + 1, cr : cr + TQ],
                    start=True,
                    stop=True,
                    tile_position=(pr, h * HD),
                )
            rq = ynp.tile([128, TQ], F32, tag="rq", name=f"rq{b}")
            nc.vector.reciprocal_approx_fast(out=rq[:], in_=rbb[:])
            yn = ynp.tile([128, TQ], BF16, tag="yn", name=f"yn{b}")
            nc.vector.tensor_mul(yn[:], y_sb[:, b * TQ : (b + 1) * TQ], rq[:])
            nc.sync.dma_start(
                out=send_t[b][:].rearrange("j p r -> p j r"),
                in_=yn[:].rearrange("p (j r) -> p j r", j=NCORES),
            )
            nc.gpsimd.collective_compute(
                "AllToAll",
                mybir.AluOpType.bypass,
                replica_groups=[list(range(NCORES))],
                ins=[send_t[b][:].opt()],
                outs=[recv_t[b][:].opt()],
            )
            if b >= 1:
                emit_recv(b - 1)

        # ---- attention: per tq block, with next-block QKV matmul groups
        # interleaved as PE filler (keeps TensorE dense so HAM stays warm).
        emit_q(0)
        emit_k(0)
        for ci in range(4):
            emit_v(ci)

        for b in range(NB):
            ts = b * TQ
            nchunks = 4 * (b + 1)
            nsc = nchunks // 2
            # filler: block b+1's QKV groups, spread across this block
            filler = []
            if b + 1 < NB:
                filler = (
                    [lambda tb=b + 1: emit_q(tb), lambda tb=b + 1: emit_k(tb)]
                    + [lambda ci=c: emit_v(ci) for c in range(4 * b + 4, 4 * b + 8)]
                )
            fidx = 0
            fevery = max(1, nsc // max(len(filler), 1))
            y0 = psY.tile([VW, TQ], F32, tag="py", name=f"y0_{b}")
            y1 = psY.tile([VW, TQ], F32, tag="py", name=f"y1_{b}")
            # diagonal chunk-pairs first so the DVE mask-mul latency hides
            # under the history chunks instead of stalling the block boundary
            sc_order = [nsc - 2, nsc - 1] + list(range(nsc - 2)) if nsc >= 2 else [0]
            for si, sc in enumerate(sc_order):
                if b > 0 and si == min(2, nsc - 1):
                    emit_norm_send(b - 1)
                # diag chunks (off > 0) only have valid scores at q >= off:
                # compute S/exp/PV on the [off, TQ) column strip and mask the
                # [off, off+CK) boundary strip with a small triangle multiply.
                offs = [max(0, (2 * sc + j) * CK - ts) for j in range(2)]
                s0 = psS.tile([128, 1024], F32, tag="ps", name=f"s0_{b}_{sc}")
                s1 = psS.tile([128, 1024], F32, tag="ps", name=f"s1_{b}_{sc}")
                for j in range(2):
                    ci = 2 * sc + j
                    off = offs[j]
                    for s_, r0 in ((s0, 0), (s1, HD)):
                        nc.tensor.matmul(
                            out=s_[:, j * TQ + off : (j + 1) * TQ],
                            lhsT=kT_sb[r0 : r0 + HD, ci * CK : (ci + 1) * CK],
                            rhs=qT_sb[r0 : r0 + HD, ts + off : ts + TQ],
                            start=True,
                            stop=True,
                        )
                p0 = pwork.tile([128, 1024], BF16, tag="pt", name=f"p0_{b}_{sc}")
                p1 = pwork.tile([128, 1024], BF16, tag="pt", name=f"p1_{b}_{sc}")
                for pt, st in ((p0, s0), (p1, s1)):
                    if offs[1] > 0:
                        for j in range(2):
                            cs = slice(j * TQ + offs[j], (j + 1) * TQ)
                            nc.scalar.activation(
                                out=pt[:, cs],
                                in_=st[:, cs],
                                func=mybir.ActivationFunctionType.Exp,
                                bias=zb_sb[:],
                            )
                    else:
                        nc.scalar.activation(
                            out=pt[:], in_=st[:],
                            func=mybir.ActivationFunctionType.Exp, bias=zb_sb[:],
                        )
                for j in range(2):
                    off = offs[j]
                    if (2 * sc + j) * CK - ts >= 0:
                        cs = slice(j * TQ + off, j * TQ + off + CK)
                        nc.vector.tensor_mul(p0[:, cs], p0[:, cs], tri_sb[:])
                        nc.vector.tensor_mul(p1[:, cs], p1[:, cs], tri_sb[:])
                for j in range(2):
                    ci = 2 * sc + j
                    off = offs[j]
                    nc.tensor.matmul(
                        out=y0[:, off:TQ],
                        lhsT=v01_sb[:, ci * 2 * VW : ci * 2 * VW + VW],
                        rhs=p0[:, j * TQ + off : (j + 1) * TQ],
                        start=(si == 0 and j == 0),
                        stop=(si == nsc - 1 and j == 1),
                    )
                    nc.tensor.matmul(
                        out=y1[:, off:TQ],
                        lhsT=v01_sb[:, ci * 2 * VW + VW : (ci + 1) * 2 * VW],
                        rhs=p1[:, j * TQ + off : (j + 1) * TQ],
                        start=(si == 0 and j == 0),
                        stop=(si == nsc - 1 and j == 1),
                    )
                if filler and (si + 1) % fevery == 0 and fidx < len(filler):
                    filler[fidx]()
                    fidx += 1
            while fidx < len(filler):
                filler[fidx]()
                fidx += 1
            # stash unnormalized y + denominator rows (normalization deferred
            # into the next block so it never stalls this pipeline)
            for h, yy in ((0, y0), (1, y1)):
                i = 2 * b + h
                nc.vector.tensor_copy(
                    out=dsp_sb[
                        (i % 4) * 32 : (i % 4) * 32 + 1,
                        (i // 4) * TQ : (i // 4 + 1) * TQ,
                    ],
                    in_=yy[HD : HD + 1, :],
                )
            nc.vector.tensor_copy(out=y_sb[0:HD, ts : ts + TQ], in_=y0[0:HD, :])
            nc.vector.tensor_copy(out=y_sb[HD:128, ts : ts + TQ], in_=y1[0:HD, :])
        emit_norm_send(NB - 1)
        emit_recv(NB - 2)

        # ---- output projection; groups 0-2 run while the last AllToAll is
        # still in flight, only group 3 (rows of blocks 6,7) is serial.
        def emit_proj(mt):
            out_sb = osb.tile([128, D], F32, tag="o", name=f"osb{mt}")
            for nh in range(2):
                po = psS.tile([128, 1024], F32, tag="ps", name=f"po{mt}_{nh}")
                for kc in range(KD):
                    nc.tensor.matmul(
                        out=po[:, 0:512],
                        lhsT=ya_mt[mt][:, kc * 128 : (kc + 1) * 128],
                        rhs=wp_sb[:, kc * D + nh * 512 : kc * D + (nh + 1) * 512],
                        start=(kc == 0),
                        stop=(kc == KD - 1),
                    )
                if nh == 0:
                    nc.scalar.copy(out=out_sb[:, 0:512], in_=po[:, 0:512])
                else:
                    nc.vector.tensor_copy(out=out_sb[:, 512:1024], in_=po[:, 0:512])
            nc.sync.dma_start(out=out[mt * 128 : (mt + 1) * 128, :], in_=out_sb[:])

        for mt in range(3):
            emit_proj(mt)
        emit_recv(NB - 1)
        emit_proj(3)

    nc.compile()
    return nc


def _inputs(x, w_attn, w_proj):
    x = np.asarray(x, dtype=np.float32).reshape(T, D)
    w_attn = np.asarray(w_attn, dtype=np.float32)
    w_proj = np.asarray(w_proj, dtype=np.float32)

    # xTb[p, tb, kc, tq] = x[tq + tb*TQ, kc*128 + p]  (block-major, contiguous loads)
    xT = x.T.reshape(KD, 128, NB, TQ)              # [kc, p, tb, tq]
    xTb_np = np.ascontiguousarray(xT.transpose(1, 2, 0, 3)).reshape(128, -1)
    xTb_np = xTb_np.astype(NPBF16)

    def wshape(w):  # [E_out rows, D] -> [128, KD*E_out] in kc-major SBUF layout
        wt = w.T.reshape(KD, 128, w.shape[0])      # [kc, p, e]
        return np.ascontiguousarray(wt.transpose(1, 0, 2)).reshape(128, -1).astype(
            NPBF16
        )

    wpT_np = wshape(w_proj)
    scale = 1.0 / math.sqrt(HD)

    # boundary triangle: tri[p, q] = 1 iff q >= p (within-chunk causality)
    pmask_np = np.triu(np.ones((128, 128), dtype=np.float32)).astype(NPBF16)

    in_maps = []
    for core in range(NCORES):
        r0 = core * E
        in_maps.append(
            {
                "xTb": xTb_np,
                "wqT": wshape(w_attn[r0 : r0 + E, :] * scale),
                "wkT": wshape(w_attn[D + r0 : D + r0 + E, :]),
                "wvT": wshape(w_attn[2 * D + r0 : 2 * D + r0 + E, :]),
                "wpT": wpT_np,
                "pmask": pmask_np,
            }
        )
    return in_maps


def kernel(x, w_attn, w_proj, _trace=False):
    if "nc" not in _CACHE:
        _CACHE["nc"] = _build()
    nc = _CACHE["nc"]
    in_maps = _inputs(x, w_attn, w_proj)
    res = run_bass_kernel_spmd(
        nc, in_maps, core_ids=list(range(NCORES)), trace=_trace
    )
    _CACHE["last_result"] = res
    # core c's out rows are (block b, strip s) pairs: full row = 512b + 64c + s
    arr = np.stack([res.results[c]["out"] for c in range(NCORES)])  # [c, b*64+s, D]
    arr = arr.reshape(NCORES, NB, RS, D).transpose(1, 0, 2, 3)      # [b, c, s, D]
    return arr.reshape(B, T, D).astype(np.float32)
